# revision 11
# baseline (speedup 1.0000x reference)
"""Trainium2 Bass kernel for nn_ContrastiveLoss (8-core SPMD).

Gather-free formulation. The baseline gathered two 512B embedding rows per
pair via SWDGE dma_gather; descriptor generation on GPSIMD (~8ns/descriptor
x 131072 descriptors/core) made it ~1.1ms. Instead, compute the FULL
8192x8192 pairwise distance matrix blockwise on the PE array and contract
it against dense pair-count matrices built host-side from the indices:

  pos_loss_sum = sum_{n,m} Cpos[n,m] * d2[n,m]
  neg_loss_sum = sum_{n,m} Cneg[n,m] * relu(bias - sqrt(d2[n,m]))^2

Each core owns a 1024-row block of d2 (8 partition-tiles of 128 rows):
  - PE: psum = Xblk^T X (K=128, bf16) accumulated with a K=1 matmul adding
    -0.5*nrm_n, so  -2*psum = -2G + nrm_n.
  - ACT: d2c = Relu(-2*psum + nrm_m)  (bias = per-partition nrm, fused clamp
    that guards sqrt against bf16-rounded negative diagonal cells)
  - ACT: d = Sqrt(d2c); m = Relu(-d + softplus_bias); m2 = Square(m)
  - DVE: tensor_tensor_reduce  acc_pos += sum(d2c * Cpos_tile),
                               acc_neg += sum(m2 * Cneg_tile)
  - Cpos/Cneg tiles stream from HBM (bf16, 32MB/core) double-buffered.
Host: build Cpos/Cneg with np.add.at (index-only preprocessing), final
f64 mean + 0.5 factors. Counts <= ~3 are exact in bf16; C_ii = 0 for both
pair types, so clamped diagonal cells never contribute.
"""

import sys

if "/opt/trn_rl_repo" not in sys.path:
    sys.path.insert(0, "/opt/trn_rl_repo")

import numpy as np
import ml_dtypes

import concourse.bass as bass
import concourse.mybir as mybir
from concourse.library_overlay import lower_extended_insts
from concourse.bass_utils import run_bass_kernel_spmd

N, D = 8192, 128
NUM_PAIRS = 262144
NCORES = 8
ROWS = N // NCORES            # 1024 rows of d2 per core
NPT = ROWS // 128             # 8 partition-tiles per core
NCH = 2048                    # columns per chunk
NCHUNK = N // NCH             # 4 chunks per partition-tile
NIT = NPT * NCHUNK            # 32 iterations per core
CSLOT = 3                     # C-tile stream slots

BF16 = ml_dtypes.bfloat16

_nc_cache = None
_last_results = None


def _build_nc():
    nc = bass.Bass()
    f32 = mybir.dt.float32
    bf = mybir.dt.bfloat16
    xt = nc.dram_tensor("xt", [128, N], bf, kind="ExternalInput")
    nrow = nc.dram_tensor("nrow", [1, N], bf, kind="ExternalInput")  # -0.5*nrm
    nrm8 = nc.dram_tensor("nrm8", [128, NPT], f32, kind="ExternalInput")
    ones1 = nc.dram_tensor("ones1", [1, 128], bf, kind="ExternalInput")
    xl = nc.dram_tensor("xl", [128, ROWS], bf, kind="ExternalInput")
    biasv = nc.dram_tensor("biasv", [128, 1], f32, kind="ExternalInput")
    cpos = nc.dram_tensor("cpos", [ROWS, N], bf, kind="ExternalInput")
    cneg = nc.dram_tensor("cneg", [ROWS, N], bf, kind="ExternalInput")
    out = nc.dram_tensor("out", [128, 2 * NIT], f32, kind="ExternalOutput")

    from contextlib import ExitStack

    with ExitStack() as ctx:
        xt_sb = ctx.enter_context(nc.sbuf_tensor("xt_sb", [128, N], bf))
        nrow_sb = ctx.enter_context(nc.sbuf_tensor("nrow_sb", [1, N], bf))
        nrm8_sb = ctx.enter_context(nc.sbuf_tensor("nrm8_sb", [128, NPT], f32))
        ones_sb = ctx.enter_context(nc.sbuf_tensor("ones_sb", [1, 128], bf))
        xl_sb = ctx.enter_context(nc.sbuf_tensor("xl_sb", [128, ROWS], bf))
        bias_sb = ctx.enter_context(nc.sbuf_tensor("bias_sb", [128, 1], f32))
        cp_sb = ctx.enter_context(nc.sbuf_tensor("cp_sb", [128, CSLOT, NCH], bf))
        cn_sb = ctx.enter_context(nc.sbuf_tensor("cn_sb", [128, CSLOT, NCH], bf))
        d2c = ctx.enter_context(nc.sbuf_tensor("d2c", [128, 2, NCH], f32))
        dd = ctx.enter_context(nc.sbuf_tensor("dd", [128, 2, NCH], f32))
        mm = ctx.enter_context(nc.sbuf_tensor("mm", [128, 2, NCH], f32))
        m2 = ctx.enter_context(nc.sbuf_tensor("m2", [128, 2, NCH], f32))
        junk = ctx.enter_context(nc.sbuf_tensor("junk", [128, NCH], f32))
        acc = ctx.enter_context(nc.sbuf_tensor("acc", [128, 2 * NIT], f32))
        ps = ctx.enter_context(nc.psum_tensor("ps", [128, 2, NCH], f32))
        s_in = ctx.enter_context(nc.semaphore("s_in"))
        s_cs = [ctx.enter_context(nc.semaphore(f"s_c{i}")) for i in range(CSLOT)]
        s_mm = ctx.enter_context(nc.semaphore("s_mm"))
        s_t = ctx.enter_context(nc.semaphore("s_t"))
        s_sq = ctx.enter_context(nc.semaphore("s_sq"))
        s_ttr = ctx.enter_context(nc.semaphore("s_ttr"))
        s_out = ctx.enter_context(nc.semaphore("s_out"))
        s_sa = ctx.enter_context(nc.semaphore("s_sa"))
        s_sv = ctx.enter_context(nc.semaphore("s_sv"))
        block = ctx.enter_context(nc.Block())

        @block.sync
        def _(sync):
            sync.dma_start(xt_sb[:], xt[:]).then_inc(s_in, 16)
            sync.dma_start(nrow_sb[:], nrow[:]).then_inc(s_in, 16)
            sync.dma_start(nrm8_sb[:], nrm8[:]).then_inc(s_in, 16)
            sync.dma_start(ones_sb[:], ones1[:]).then_inc(s_in, 16)
            sync.dma_start(bias_sb[:], biasv[:]).then_inc(s_in, 16)
            sync.dma_start(xl_sb[:], xl[:]).then_inc(s_in, 16)
            for k in range(NIT):
                pt, ch = divmod(k, NCHUNK)
                s = k % CSLOT
                rs = slice(pt * 128, (pt + 1) * 128)
                cs = slice(ch * NCH, (ch + 1) * NCH)
                if k >= CSLOT:
                    sync.wait_ge(s_ttr, k - CSLOT + 1)
                sync.dma_start(cp_sb[:, s], cpos[rs, cs]).then_inc(s_cs[s], 16)
                sync.dma_start(cn_sb[:, s], cneg[rs, cs]).then_inc(s_cs[s], 16)
            sync.wait_ge(s_ttr, NIT)
            sync.dma_start(out[:], acc[:]).then_inc(s_out, 16)
            sync.wait_ge(s_out, 16)

        @block.tensor
        def _(tensor):
            tensor.wait_ge(s_in, 16 * 6)
            for k in range(NIT):
                pt, ch = divmod(k, NCHUNK)
                r = k % 2
                if k >= 2:
                    tensor.wait_ge(s_t, k - 1)
                for c in range(NCH // 512):
                    c0 = ch * NCH + c * 512
                    tensor.matmul(
                        ps[:, r, c * 512 : (c + 1) * 512],
                        xl_sb[:, pt * 128 : (pt + 1) * 128],
                        xt_sb[:, c0 : c0 + 512],
                        start=True,
                        stop=False,
                    )
                    tensor.matmul(
                        ps[:, r, c * 512 : (c + 1) * 512],
                        ones_sb[:, :],
                        nrow_sb[:, c0 : c0 + 512],
                        start=False,
                        stop=True,
                    ).then_inc(s_mm, 1)

        @block.scalar
        def _(scalar):
            nmm = NCH // 512
            for k in range(NIT):
                pt, ch = divmod(k, NCHUNK)
                r = k % 2
                if k >= 2:
                    scalar.wait_ge(s_ttr, k - 1)
                scalar.wait_ge(s_mm, nmm * min(NIT, k + 2))
                scalar.activation(
                    d2c[:, r],
                    ps[:, r],
                    mybir.ActivationFunctionType.Relu,
                    bias=nrm8_sb[:, pt : pt + 1],
                    scale=-2.0,
                ).then_inc(s_t, 1)
                scalar.wait_ge(s_t, k + 1)
                scalar.activation(
                    dd[:, r], d2c[:, r], mybir.ActivationFunctionType.Sqrt
                ).then_inc(s_sa, 1)
                scalar.wait_ge(s_sa, 2 * k + 1)
                scalar.activation(
                    mm[:, r],
                    dd[:, r],
                    mybir.ActivationFunctionType.Relu,
                    bias=bias_sb[:, 0:1],
                    scale=-1.0,
                ).then_inc(s_sa, 1)
                scalar.wait_ge(s_sa, 2 * k + 2)
                scalar.activation(
                    m2[:, r], mm[:, r], mybir.ActivationFunctionType.Square
                ).then_inc(s_sq, 1)

        @block.vector
        def _(vector):
            for k in range(NIT):
                r = k % 2
                s = k % CSLOT
                if k >= 1:
                    vector.wait_ge(s_ttr, k)
                vector.wait_ge(s_cs[k % CSLOT], 32 * (k // CSLOT + 1))
                vector.wait_ge(s_t, k + 1)
                vector.tensor_tensor(
                    junk[:], d2c[:, r], cp_sb[:, s], op=mybir.AluOpType.mult
                ).then_inc(s_sv, 1)
                vector.wait_ge(s_sv, 3 * k + 1)
                vector.tensor_reduce(
                    acc[:, k : k + 1],
                    junk[:],
                    axis=mybir.AxisListType.X,
                    op=mybir.AluOpType.add,
                ).then_inc(s_sv, 1)
                vector.wait_ge(s_sv, 3 * k + 2)
                vector.wait_ge(s_sq, k + 1)
                vector.tensor_tensor(
                    junk[:], m2[:, r], cn_sb[:, s], op=mybir.AluOpType.mult
                ).then_inc(s_sv, 1)
                vector.wait_ge(s_sv, 3 * k + 3)
                vector.tensor_reduce(
                    acc[:, NIT + k : NIT + k + 1],
                    junk[:],
                    axis=mybir.AxisListType.X,
                    op=mybir.AluOpType.add,
                ).then_inc(s_ttr, 1)

    lower_extended_insts(nc)
    return nc


def _get_nc():
    global _nc_cache
    if _nc_cache is None:
        _nc_cache = _build_nc()
    return _nc_cache


def kernel(**inputs):
    global _last_results
    X = np.ascontiguousarray(np.asarray(inputs["Xemb"], dtype=np.float32))
    h_bias = float(np.asarray(inputs["h_bias"]))
    pos_idx = np.asarray(inputs["pos_idx"], dtype=np.int64)
    neg_idx = np.asarray(inputs["neg_idx"], dtype=np.int64)

    nrm = (X.astype(np.float64) ** 2).sum(axis=1)  # [N]
    xt_bf = np.ascontiguousarray(X.T).astype(BF16)  # [128, N]
    nrow_bf = (-0.5 * nrm[None, :]).astype(BF16)  # [1, N]
    ones_bf = np.ones((1, 128), dtype=BF16)
    softplus = float(np.logaddexp(0.0, h_bias))
    bias_col = np.full((128, 1), softplus, dtype=np.float32)

    cpos = np.zeros((N, N), dtype=np.float32)
    np.add.at(cpos, (pos_idx[:, 0], pos_idx[:, 1]), 1.0)
    cpos = cpos.astype(BF16)
    cneg = np.zeros((N, N), dtype=np.float32)
    np.add.at(cneg, (neg_idx[:, 0], neg_idx[:, 1]), 1.0)
    cneg = cneg.astype(BF16)

    in_maps = []
    for c in range(NCORES):
        rs = slice(c * ROWS, (c + 1) * ROWS)
        nrm8 = np.ascontiguousarray(
            nrm[rs].astype(np.float32).reshape(NPT, 128).T
        )  # [128, NPT]
        in_maps.append(
            {
                "xt": xt_bf,
                "xl": np.ascontiguousarray(xt_bf[:, rs]),
                "nrow": nrow_bf,
                "nrm8": nrm8,
                "ones1": ones_bf,
                "biasv": bias_col,
                "cpos": np.ascontiguousarray(cpos[rs]),
                "cneg": np.ascontiguousarray(cneg[rs]),
            }
        )

    res = run_bass_kernel_spmd(_get_nc(), in_maps, core_ids=list(range(NCORES)))
    _last_results = res

    pos_sum = 0.0
    neg_sum = 0.0
    for c in range(NCORES):
        o = np.asarray(res.results[c]["out"], dtype=np.float64)
        pos_sum += o[:, :NIT].sum()
        neg_sum += o[:, NIT:].sum()

    pos_loss = 0.5 * pos_sum / NUM_PAIRS
    neg_loss = 0.5 * neg_sum / NUM_PAIRS
    return np.array([pos_loss, neg_loss], dtype=np.float32)


# revision 12
# speedup vs baseline: 1.0657x; 1.0657x over previous
"""Trainium2 Bass kernel for nn_ContrastiveLoss (8-core SPMD).

Gather-free formulation. The baseline gathered two 512B embedding rows per
pair via SWDGE dma_gather; descriptor generation on GPSIMD (~8ns/descriptor
x 131072 descriptors/core) made it ~1.1ms. Instead, compute the FULL
8192x8192 pairwise distance matrix blockwise on the PE array and contract
it against dense pair-count matrices built host-side from the indices:

  pos_loss_sum = sum_{n,m} Cpos[n,m] * d2[n,m]
  neg_loss_sum = sum_{n,m} Cneg[n,m] * relu(bias - sqrt(d2[n,m]))^2

Each core owns a 1024-row block of d2 (8 partition-tiles of 128 rows):
  - PE: psum = Xblk^T X (K=128, bf16) accumulated with a K=1 matmul adding
    -0.5*nrm_n, so  -2*psum = -2G + nrm_n.
  - ACT: d2c = Relu(-2*psum + nrm_m)  (bias = per-partition nrm, fused clamp
    that guards sqrt against bf16-rounded negative diagonal cells)
  - ACT: d = Sqrt(d2c); m = Relu(-d + softplus_bias); m2 = Square(m)
  - DVE: tensor_tensor_reduce  acc_pos += sum(d2c * Cpos_tile),
                               acc_neg += sum(m2 * Cneg_tile)
  - Cpos/Cneg tiles stream from HBM (bf16, 32MB/core) double-buffered.
Host: build Cpos/Cneg with np.add.at (index-only preprocessing), final
f64 mean + 0.5 factors. Counts <= ~3 are exact in bf16; C_ii = 0 for both
pair types, so clamped diagonal cells never contribute.
"""

import sys

if "/opt/trn_rl_repo" not in sys.path:
    sys.path.insert(0, "/opt/trn_rl_repo")

import numpy as np
import ml_dtypes

import concourse.bass as bass
import concourse.mybir as mybir
from concourse.library_overlay import lower_extended_insts
from concourse.bass_utils import run_bass_kernel_spmd

N, D = 8192, 128
NUM_PAIRS = 262144
NCORES = 8
ROWS = N // NCORES            # 1024 rows of d2 per core
NPT = ROWS // 128             # 8 partition-tiles per core
NCH = 2048                    # columns per chunk
NCHUNK = N // NCH             # 4 chunks per partition-tile
NIT = NPT * NCHUNK            # 32 iterations per core
CSLOT = 3                     # C-tile stream slots

BF16 = ml_dtypes.bfloat16

_nc_cache = None
_last_results = None


def _build_nc():
    nc = bass.Bass()
    f32 = mybir.dt.float32
    bf = mybir.dt.bfloat16
    xt = nc.dram_tensor("xt", [128, N], bf, kind="ExternalInput")
    nrow = nc.dram_tensor("nrow", [1, N], bf, kind="ExternalInput")  # -0.5*nrm
    nrm8 = nc.dram_tensor("nrm8", [128, NPT], f32, kind="ExternalInput")
    ones1 = nc.dram_tensor("ones1", [1, 128], bf, kind="ExternalInput")
    xl = nc.dram_tensor("xl", [128, ROWS], bf, kind="ExternalInput")
    biasv = nc.dram_tensor("biasv", [128, 1], f32, kind="ExternalInput")
    cpos = nc.dram_tensor("cpos", [ROWS, N], bf, kind="ExternalInput")
    cneg = nc.dram_tensor("cneg", [ROWS, N], bf, kind="ExternalInput")
    out = nc.dram_tensor("out", [128, 2 * NIT * 16], f32, kind="ExternalOutput")

    from contextlib import ExitStack

    with ExitStack() as ctx:
        xt_sb = ctx.enter_context(nc.sbuf_tensor("xt_sb", [128, N], bf))
        nrow_sb = ctx.enter_context(nc.sbuf_tensor("nrow_sb", [1, N], bf))
        nrm8_sb = ctx.enter_context(nc.sbuf_tensor("nrm8_sb", [128, NPT], f32))
        ones_sb = ctx.enter_context(nc.sbuf_tensor("ones_sb", [1, 128], bf))
        xl_sb = ctx.enter_context(nc.sbuf_tensor("xl_sb", [128, ROWS], bf))
        bias_sb = ctx.enter_context(nc.sbuf_tensor("bias_sb", [128, 1], f32))
        cp_sb = ctx.enter_context(nc.sbuf_tensor("cp_sb", [128, CSLOT, 16, 128], bf))
        cn_sb = ctx.enter_context(nc.sbuf_tensor("cn_sb", [128, CSLOT, 16, 128], bf))
        d2c = ctx.enter_context(nc.sbuf_tensor("d2c", [128, 2, 16, 128], bf))
        dd = ctx.enter_context(nc.sbuf_tensor("dd", [128, 2, 16, 128], bf))
        mm = ctx.enter_context(nc.sbuf_tensor("mm", [128, 2, 16, 128], bf))
        m2 = ctx.enter_context(nc.sbuf_tensor("m2", [128, 2, 16, 128], bf))
        junk = ctx.enter_context(nc.sbuf_tensor("junk", [128, 16, 128], bf))
        acc = ctx.enter_context(nc.sbuf_tensor("acc", [128, 2 * NIT * 16], f32))
        ps = ctx.enter_context(nc.psum_tensor("ps", [128, 2, 16, 128], f32))
        s_in = ctx.enter_context(nc.semaphore("s_in"))
        s_cs = [ctx.enter_context(nc.semaphore(f"s_c{i}")) for i in range(CSLOT)]
        s_mm = ctx.enter_context(nc.semaphore("s_mm"))
        s_t = ctx.enter_context(nc.semaphore("s_t"))
        s_sq = ctx.enter_context(nc.semaphore("s_sq"))
        s_ttr = ctx.enter_context(nc.semaphore("s_ttr"))
        s_out = ctx.enter_context(nc.semaphore("s_out"))
        s_sa = ctx.enter_context(nc.semaphore("s_sa"))
        s_sv = ctx.enter_context(nc.semaphore("s_sv"))
        block = ctx.enter_context(nc.Block())

        @block.sync
        def _(sync):
            sync.dma_start(xt_sb[:], xt[:]).then_inc(s_in, 16)
            sync.dma_start(nrow_sb[:], nrow[:]).then_inc(s_in, 16)
            sync.dma_start(nrm8_sb[:], nrm8[:]).then_inc(s_in, 16)
            sync.dma_start(ones_sb[:], ones1[:]).then_inc(s_in, 16)
            sync.dma_start(bias_sb[:], biasv[:]).then_inc(s_in, 16)
            sync.dma_start(xl_sb[:], xl[:]).then_inc(s_in, 16)
            for k in range(NIT):
                pt, ch = divmod(k, NCHUNK)
                s = k % CSLOT
                rs = slice(pt * 128, (pt + 1) * 128)
                cs = slice(ch * NCH, (ch + 1) * NCH)
                if k >= CSLOT:
                    sync.wait_ge(s_ttr, k - CSLOT + 1)
                sync.dma_start(cp_sb[:, s], cpos[rs, cs]).then_inc(s_cs[s], 16)
                sync.dma_start(cn_sb[:, s], cneg[rs, cs]).then_inc(s_cs[s], 16)
            sync.wait_ge(s_ttr, NIT)
            sync.dma_start(out[:], acc[:]).then_inc(s_out, 16)
            sync.wait_ge(s_out, 16)

        @block.tensor
        def _(tensor):
            tensor.wait_ge(s_in, 16 * 6)
            for k in range(NIT):
                pt, ch = divmod(k, NCHUNK)
                r = k % 2
                if k >= 2:
                    tensor.wait_ge(s_t, k - 1)
                for c in range(NCH // 512):
                    c0 = ch * NCH + c * 512
                    tensor.matmul(
                        ps[:, r, 4 * c : 4 * c + 4, :],
                        xl_sb[:, pt * 128 : (pt + 1) * 128],
                        xt_sb[:, c0 : c0 + 512],
                        start=True,
                        stop=False,
                    )
                    tensor.matmul(
                        ps[:, r, 4 * c : 4 * c + 4, :],
                        ones_sb[:, :],
                        nrow_sb[:, c0 : c0 + 512],
                        start=False,
                        stop=True,
                    ).then_inc(s_mm, 1)

        @block.scalar
        def _(scalar):
            nmm = NCH // 512
            for k in range(NIT):
                pt, ch = divmod(k, NCHUNK)
                r = k % 2
                if k >= 2:
                    scalar.wait_ge(s_ttr, k - 1)
                scalar.wait_ge(s_mm, nmm * min(NIT, k + 2))
                scalar.activation(
                    d2c[:, r],
                    ps[:, r],
                    mybir.ActivationFunctionType.Relu,
                    bias=nrm8_sb[:, pt : pt + 1],
                    scale=-2.0,
                ).then_inc(s_t, 1)
                scalar.wait_ge(s_t, k + 1)
                scalar.activation(
                    dd[:, r], d2c[:, r], mybir.ActivationFunctionType.Sqrt
                ).then_inc(s_sa, 1)
                scalar.wait_ge(s_sa, k + 1)
                scalar.activation(
                    mm[:, r],
                    dd[:, r],
                    mybir.ActivationFunctionType.Relu,
                    bias=bias_sb[:, 0:1],
                    scale=-1.0,
                ).then_inc(s_sq, 1)

        @block.vector
        def _(vector):
            for k in range(NIT):
                r = k % 2
                cslot = k % CSLOT
                if k >= 1:
                    vector.wait_ge(s_ttr, k)
                vector.wait_ge(s_cs[cslot], 32 * (k // CSLOT + 1))
                vector.wait_ge(s_t, k + 1)
                vector.tensor_tensor(
                    junk[:], d2c[:, r], cp_sb[:, cslot], op=mybir.AluOpType.mult
                ).then_inc(s_sv, 1)
                vector.wait_ge(s_sv, 4 * k + 1)
                vector.tensor_reduce(
                    acc[:, 16 * k : 16 * (k + 1)],
                    junk[:],
                    axis=mybir.AxisListType.X,
                    op=mybir.AluOpType.add,
                ).then_inc(s_sv, 1)
                vector.wait_ge(s_sv, 4 * k + 2)
                vector.wait_ge(s_sq, k + 1)
                vector.tensor_tensor(
                    m2[:, r], mm[:, r], mm[:, r], op=mybir.AluOpType.mult
                ).then_inc(s_sv, 1)
                vector.wait_ge(s_sv, 4 * k + 3)
                vector.tensor_tensor(
                    junk[:], m2[:, r], cn_sb[:, cslot], op=mybir.AluOpType.mult
                ).then_inc(s_sv, 1)
                vector.wait_ge(s_sv, 4 * k + 4)
                vector.tensor_reduce(
                    acc[:, 16 * (NIT + k) : 16 * (NIT + k + 1)],
                    junk[:],
                    axis=mybir.AxisListType.X,
                    op=mybir.AluOpType.add,
                ).then_inc(s_ttr, 1)

    lower_extended_insts(nc)
    return nc


def _get_nc():
    global _nc_cache
    if _nc_cache is None:
        _nc_cache = _build_nc()
    return _nc_cache


def kernel(**inputs):
    global _last_results
    X = np.ascontiguousarray(np.asarray(inputs["Xemb"], dtype=np.float32))
    h_bias = float(np.asarray(inputs["h_bias"]))
    pos_idx = np.asarray(inputs["pos_idx"], dtype=np.int64)
    neg_idx = np.asarray(inputs["neg_idx"], dtype=np.int64)

    nrm = (X.astype(np.float64) ** 2).sum(axis=1)  # [N]
    xt_bf = np.ascontiguousarray(X.T).astype(BF16)  # [128, N]
    nrow_bf = (-0.5 * nrm[None, :]).astype(BF16)  # [1, N]
    ones_bf = np.ones((1, 128), dtype=BF16)
    softplus = float(np.logaddexp(0.0, h_bias))
    bias_col = np.full((128, 1), softplus, dtype=np.float32)

    cpos = np.zeros((N, N), dtype=np.float32)
    np.add.at(cpos, (pos_idx[:, 0], pos_idx[:, 1]), 1.0)
    cpos = cpos.astype(BF16)
    cneg = np.zeros((N, N), dtype=np.float32)
    np.add.at(cneg, (neg_idx[:, 0], neg_idx[:, 1]), 1.0)
    cneg = cneg.astype(BF16)

    in_maps = []
    for c in range(NCORES):
        rs = slice(c * ROWS, (c + 1) * ROWS)
        nrm8 = np.ascontiguousarray(
            nrm[rs].astype(np.float32).reshape(NPT, 128).T
        )  # [128, NPT]
        in_maps.append(
            {
                "xt": xt_bf,
                "xl": np.ascontiguousarray(xt_bf[:, rs]),
                "nrow": nrow_bf,
                "nrm8": nrm8,
                "ones1": ones_bf,
                "biasv": bias_col,
                "cpos": np.ascontiguousarray(cpos[rs]),
                "cneg": np.ascontiguousarray(cneg[rs]),
            }
        )

    res = run_bass_kernel_spmd(_get_nc(), in_maps, core_ids=list(range(NCORES)))
    _last_results = res

    pos_sum = 0.0
    neg_sum = 0.0
    for c in range(NCORES):
        o = np.asarray(res.results[c]["out"], dtype=np.float64)
        pos_sum += o[:, : NIT * 16].sum()
        neg_sum += o[:, NIT * 16 :].sum()

    pos_loss = 0.5 * pos_sum / NUM_PAIRS
    neg_loss = 0.5 * neg_sum / NUM_PAIRS
    return np.array([pos_loss, neg_loss], dtype=np.float32)
